# revision 29
# baseline (speedup 1.0000x reference)
"""MBD degradation-imputation sampling step on 8 Trainium2 NeuronCores.

Strategy (data-parallel over the N=2048 candidate samples, 256/core):
  pass A : per-sample consistency scores, one HBM pass over eps.
           Observed positions are made sample-independent by a
           host-prepared additive tensor c0 that saturates the clip
           (softmax is shift-invariant so the constant contribution
           cancels):
               u  = eps + c0            (DVE tensor_tensor, f32)
               vh = clip(u, +-1/sigma)  (DVE tensor_scalar -> fp16 CACHE)
               d  = vh - q'             (Pool tensor_tensor, fp16)
               S += sum(d^2)            (ACT Square + accum_out)
           The fp16 clipped values stay resident in SBUF (128 KiB/pn)
           so pass B never re-reads eps.
  AllGather the 2048 scores (8 KB), softmax stats on-device, each core
  weights its local samples.
  pass B : weighted partition-reduction straight out of the fp16 SBUF
           cache on the TensorEngine (fp16 matmuls, M=1, PSUM-
           accumulated), AllReduce the (T,F) partials (128 KB), final
           mask-select against observed_data.

`stage` truncates the program for hardware bisection:
  1 = pass A only, 2 = +AllGather/softmax, 3 = +pass B (no AllReduce),
  4 = full kernel.
"""

from contextlib import ExitStack

import numpy as np

import concourse.bass as bass
import concourse.tile as tile
from concourse import bacc, mybir
from concourse.bass_utils import run_bass_kernel_spmd

N_CORES = 8
N, T, F = 2048, 512, 64
P = 128
TF = T * F                      # 32768
NLOC = N // N_CORES             # 256
NBLK = NLOC // P                # 2
CHUNK = 1024
NCHUNK = TF // CHUNK            # 32
SUB = 512                       # matmul N (one PSUM bank)
TEMP = 0.1
T_STEPS = 1000

F32 = mybir.dt.float32
F16 = mybir.dt.float16
AX = mybir.AxisListType
ALU = mybir.AluOpType
ACTF = mybir.ActivationFunctionType


def _schedule_scalars(i: int):
    s = 0.008
    x = np.linspace(0, T_STEPS, T_STEPS + 1, dtype=np.float64)
    ac = np.cos((x / T_STEPS + s) / (1 + s) * np.pi * 0.5) ** 2
    ac = ac / ac[0]
    betas = np.clip(1.0 - ac[1:] / ac[:-1], 0.0, 0.999)
    alphas = 1.0 - betas
    acp = np.cumprod(alphas)
    abar_i = np.float32(acp[i])
    sigma_i = np.float32(np.sqrt(1.0 - acp[i]))
    alpha_i = np.float32(alphas[i])
    abar_im1 = np.float32(acp[i - 1])
    sa = np.float32(np.sqrt(abar_i))
    # the reference's Yi terms cancel exactly; out_missing = c1 * weighted
    c1 = np.float32(sa / np.float32(np.sqrt(alpha_i)) / np.float32(np.sqrt(abar_im1)))
    return sigma_i, c1


def _build(sigma_i: float, c1: float, stage: int = 4):
    inv_sig = float(np.float32(1.0 / np.float32(sigma_i)))
    sigma_i = float(np.float32(sigma_i))
    c1 = float(np.float32(c1))
    # scores = cA * sum((v - q')^2)  (+ sample-independent shift vs ref)
    cA = float(np.float32(-(np.float32(sigma_i) ** 2) / np.float32(TF)))

    nc = bacc.Bacc(
        "TRN2", target_bir_lowering=False, debug=False, num_devices=N_CORES
    )
    eps_d = nc.dram_tensor("eps", [NLOC, TF], F32, kind="ExternalInput")
    c0_d = nc.dram_tensor("c0", [TF], F32, kind="ExternalInput")
    qp_d = nc.dram_tensor("qp", [TF], F16, kind="ExternalInput")
    obs_d = nc.dram_tensor("obs", [TF], F32, kind="ExternalInput")
    maskf_d = nc.dram_tensor("maskf", [TF], F32, kind="ExternalInput")
    out_d = nc.dram_tensor("out", [TF], F32, kind="ExternalOutput")

    ones_d = nc.dram_tensor("ones", [P], F32, kind="ExternalInput")
    sc_loc_d = nc.dram_tensor("sc_loc", [NLOC], F32)
    sc_all_d = nc.dram_tensor("sc_all", [N], F32, addr_space="Shared")
    # ws carries the TF weighted partials plus the local softmax
    # normalizer Z in slot TF — one AllReduce delivers both.
    ws_loc_d = nc.dram_tensor("ws_loc", [TF + 4], F32)
    ws_all_d = nc.dram_tensor("ws_all", [TF + 4], F32, addr_space="Shared")

    rg = [list(range(N_CORES))]

    with tile.TileContext(nc) as tc, ExitStack() as ctx:
        eps_ap = eps_d.ap()

        rowsq = ctx.enter_context(tc.tile_pool(name="rowsq", bufs=4))
        work = ctx.enter_context(tc.tile_pool(name="work", bufs=4))
        workh = ctx.enter_context(tc.tile_pool(name="workh", bufs=2))
        cache = ctx.enter_context(tc.tile_pool(name="cache", bufs=1))
        stat = ctx.enter_context(tc.tile_pool(name="stat", bufs=1))
        smal = ctx.enter_context(tc.tile_pool(name="smal", bufs=1))
        psum = ctx.enter_context(tc.tile_pool(name="psum", bufs=1, space="PSUM"))

        # fp16 clipped-values cache: 64 tiles of [128, 1024] packed into
        # one persistent tile (128 KiB per partition)
        vcache = cache.tile([P, NBLK * NCHUNK * CHUNK], F16, tag="vc",
                            name="vcache")

        # ---------------- pass A: local scores ----------------
        # block-major so block 0's scores can AllGather while block 1
        # computes.  u = c0 + eps is built entirely by DMA engines:
        # ACT-queue broadcast fill of c0, then gpsimd DMA-accumulate eps.
        sa_cols = [
            stat.tile([P, NCHUNK], F32, tag=f"sa{b}", name=f"sa_cols{b}")
            for b in range(NBLK)
        ]
        sb_cols = [
            stat.tile([P, NCHUNK], F32, tag=f"sb{b}", name=f"sb_cols{b}")
            for b in range(NBLK)
        ]
        s_loc = stat.tile([P, NBLK], F32, tag="sloc", name="s_loc")
        for b in range(NBLK):
            for k in range(NCHUNK):
                sl = slice(k * CHUNK, (k + 1) * CHUNK)
                q_t = rowsq.tile([P, CHUNK], F16, tag="q", name="q_t")
                nc.gpsimd.dma_start(
                    out=q_t[:], in_=qp_d.ap()[sl].partition_broadcast(P)
                )
                c0_t = rowsq.tile([P, CHUNK], F32, tag="c0", name="c0_t")
                nc.scalar.dma_start(
                    out=c0_t[:], in_=c0_d.ap()[sl].partition_broadcast(P)
                )
                u_t = work.tile([P, CHUNK], F32, tag="u", name="u_t")
                nc.sync.dma_start(out=u_t[:], in_=eps_ap[b * P:(b + 1) * P, sl])
                nc.vector.scalar_tensor_tensor(
                    out=u_t[:], in0=u_t[:], scalar=0.0, in1=c0_t[:],
                    op0=ALU.add, op1=ALU.add,
                )
                off = (k * NBLK + b) * CHUNK
                vsl = vcache[:, off:off + CHUNK]
                nc.vector.tensor_scalar(
                    out=vsl, in0=u_t[:], scalar1=inv_sig, scalar2=-inv_sig,
                    op0=ALU.min, op1=ALU.max,
                )
                d_t = workh.tile([P, CHUNK], F16, tag="d", name="d_t")
                nc.vector.scalar_tensor_tensor(
                    out=d_t[:], in0=vsl, scalar=1.0, in1=q_t[:],
                    op0=ALU.mult, op1=ALU.mult,
                    accum_out=sb_cols[b][:, k:k + 1],
                )
                d2 = workh.tile([P, CHUNK], F16, tag="d", name="d2")
                nc.scalar.activation(
                    out=d2[:], in_=vsl, func=ACTF.Square,
                    accum_out=sa_cols[b][:, k:k + 1],
                )
            # block-b scores -> DRAM -> AllGather (overlaps next block)
            sa_tot = smal.tile([P, 1], F32, tag="sat", name="sa_tot")
            nc.vector.tensor_reduce(sa_tot[:], sa_cols[b][:], axis=AX.X, op=ALU.add)
            sb_tot = smal.tile([P, 1], F32, tag="sbt", name="sb_tot")
            nc.vector.tensor_reduce(sb_tot[:], sb_cols[b][:], axis=AX.X, op=ALU.add)
            dtot = smal.tile([P, 1], F32, tag="dtot", name="dtot")
            nc.vector.scalar_tensor_tensor(
                out=dtot[:], in0=sb_tot[:], scalar=-2.0, in1=sa_tot[:],
                op0=ALU.mult, op1=ALU.add,
            )
            nc.vector.tensor_scalar_mul(s_loc[:, b:b + 1], dtot[:], cA)
            nc.sync.dma_start(
                out=sc_loc_d.ap()[b * P:(b + 1) * P]
                .rearrange("(a p) -> p a", a=1),
                in_=s_loc[:, b:b + 1],
            )
            if stage >= 2:
                nc.gpsimd.collective_compute(
                    "AllGather", ALU.bypass,
                    ins=[sc_loc_d.ap()[b * P:(b + 1) * P]],
                    outs=[sc_all_d.ap()[b * P * N_CORES:(b + 1) * P * N_CORES]],
                    replica_groups=rg,
                )
        if stage <= 1:
            nc.sync.dma_start(
                out=out_d.ap()[0:NLOC].rearrange("(b p) -> p b", p=P),
                in_=s_loc[:],
            )

        # ---------------- softmax stats ----------------
        # weights are UN-normalized exp() here; the global Z rides the
        # AllReduce (slot TF of ws) and division happens post-reduce.
        wt16 = None
        if stage >= 2:
            onesr = smal.tile([1, P], F32, tag="onesr", name="onesr")
            nc.sync.dma_start(
                out=onesr[:], in_=ones_d.ap().rearrange("(a n) -> a n", a=1)
            )
            onec = smal.tile([P, 1], F32, tag="onec", name="onec")
            nc.sync.dma_start(
                out=onec[:], in_=ones_d.ap().rearrange("(p a) -> p a", a=1)
            )
            s_all = smal.tile([1, N], F32, tag="sall", name="s_all")
            nc.sync.dma_start(
                out=s_all[:], in_=sc_all_d.ap().rearrange("(a n) -> a n", a=1)
            )
            pack = smal.tile([1, 2], F32, tag="pack", name="pack")
            negmean = smal.tile([1, 1], F32, tag="negmean", name="negmean")
            nc.vector.tensor_reduce(negmean[:], s_all[:], axis=AX.X, op=ALU.add)
            nc.vector.tensor_scalar_mul(negmean[:], negmean[:], -1.0 / N)
            js = smal.tile([1, N], F16, tag="js", name="js")
            ssq = smal.tile([1, 1], F32, tag="ssq", name="ssq")
            nc.scalar.activation(
                out=js[:], in_=s_all[:], func=ACTF.Square, bias=negmean[:],
                accum_out=ssq[:],
            )
            # std = max(sqrt(ssq/(N-1)), 1e-4); pack0 = 1/(std*TEMP)
            std = smal.tile([1, 1], F32, tag="std", name="std")
            nc.scalar.activation(
                out=std[:], in_=ssq[:], func=ACTF.Sqrt, scale=1.0 / (N - 1)
            )
            stdT = smal.tile([1, 1], F32, tag="stdT", name="stdT")
            nc.vector.tensor_scalar(
                out=stdT[:], in0=std[:], scalar1=1e-4, scalar2=TEMP,
                op0=ALU.max, op1=ALU.mult,
            )
            nc.vector.reciprocal(pack[:, 0:1], stdT[:])
            mx = smal.tile([1, 1], F32, tag="mx", name="mx")
            nc.vector.tensor_reduce(mx[:], s_all[:], axis=AX.X, op=ALU.max)
            # shifted logit: (s - mx)*inv10 (mean cancels in the shift, and
            # the un-normalized exp is safe: max exponent is exactly 0)
            nmx = smal.tile([1, 1], F32, tag="nmx", name="nmx")
            nc.vector.tensor_scalar_mul(nmx[:], mx[:], -1.0)
            nc.vector.tensor_tensor(pack[:, 1:2], nmx[:], pack[:, 0:1], ALU.mult)
            # PE-broadcast (inv10, bg) to all 128 partitions
            bps = psum.tile([P, 2], F32, tag="bps", bufs=1, name="bps")
            nc.tensor.matmul(bps[:], lhsT=onesr[:], rhs=pack[:], start=True,
                             stop=True)
            scal = smal.tile([P, 2], F32, tag="scal", name="scal")
            nc.vector.tensor_copy(scal[:], bps[:])

            e_loc = smal.tile([P, NBLK], F32, tag="eloc", name="e_loc")
            nc.scalar.activation(
                out=e_loc[:], in_=s_loc[:], func=ACTF.Exp,
                scale=scal[:, 0:1], bias=scal[:, 1:2],
            )
            wt16 = stat.tile([P, NBLK], F16, tag="wt16", name="wt16")
            zloc = smal.tile([P, 1], F32, tag="zloc", name="zloc")
            nc.scalar.activation(
                out=wt16[:], in_=e_loc[:], func=ACTF.Copy, accum_out=zloc[:]
            )
            # local Z -> ws_loc[TF] so the AllReduce sums it globally
            zpt = psum.tile([P, 1], F32, tag="qps", bufs=1, name="zpt")
            zps = zpt[0:1, 0:1]
            nc.tensor.matmul(zps, lhsT=zloc[:], rhs=onec[:], start=True,
                             stop=True)
            ztot = smal.tile([1, 1], F32, tag="ztot", name="ztot")
            nc.vector.tensor_copy(ztot[:], zps[:])
            nc.sync.dma_start(
                out=ws_loc_d.ap()[TF:TF + 1].rearrange("(a n) -> a n", a=1),
                in_=ztot[:],
            )
            if stage <= 2:
                nc.sync.dma_start(
                    out=out_d.ap()[0:NLOC].rearrange("(b p) -> p b", p=P),
                    in_=e_loc[:],
                )

        # ---------------- pass B: weighted sum on PE from SBUF cache ----
        if stage >= 3:
            # warm the PE p-state during the stats window: dummy matmuls
            # gated on a post-gather tile so they run right before pass B
            jl = smal.tile([P, 1], F16, tag="jl", name="jl")
            nc.scalar.copy(jl[:], s_loc[:, 0:1])
            for w in range(10):
                wps = psum.tile([1, SUB], F32, tag="wrow", bufs=6, name="wps")
                nc.tensor.matmul(
                    wps[:], lhsT=jl[:], rhs=vcache[:, w * SUB:(w + 1) * SUB],
                    start=True, stop=True,
                )
            GRP = 2
            for s in range(TF // SUB):
                k, half = s // 2, s % 2
                wrow = psum.tile([1, SUB], F32, tag="wrow", bufs=6, name="wrow")
                for b in range(NBLK):
                    off = (k * NBLK + b) * CHUNK + half * SUB
                    nc.tensor.matmul(
                        wrow[:], lhsT=wt16[:, b:b + 1],
                        rhs=vcache[:, off:off + SUB],
                        start=(b == 0), stop=(b == NBLK - 1),
                    )
                g, gi = s // GRP, s % GRP
                if gi == 0:
                    wsb = work.tile([1, GRP * SUB], F32, tag="wsb", bufs=2,
                                    name="wsb")
                if s % 2 == 0:
                    nc.vector.tensor_copy(
                        wsb[:, gi * SUB:(gi + 1) * SUB], wrow[:]
                    )
                else:
                    nc.scalar.copy(wsb[:, gi * SUB:(gi + 1) * SUB], wrow[:])
                if gi == GRP - 1:
                    nc.sync.dma_start(
                        out=ws_loc_d.ap()[g * GRP * SUB:(g + 1) * GRP * SUB]
                        .rearrange("(a n) -> a n", a=1),
                        in_=wsb[:],
                    )
            if stage <= 3:
                o3 = stat.tile([P, TF // P], F32, tag="o3", name="o3")
                nc.sync.dma_start(
                    out=o3[:],
                    in_=ws_loc_d.ap()[0:TF].rearrange("(p c) -> p c", p=P),
                )
                nc.sync.dma_start(
                    out=out_d.ap().rearrange("(p c) -> p c", p=P), in_=o3[:]
                )

        # ---------------- AllReduce + final combine ----------------
        if stage >= 4:
            # obs/mask preloads don't depend on anything — issue early is
            # handled by the scheduler; they're plain loads.
            rowmaj0 = lambda d: d.ap()[0:TF].rearrange("(p c) -> p c", p=P)
            obs_t = stat.tile([P, TF // P], F32, tag="obsf", name="obs_t")
            nc.sync.dma_start(out=obs_t[:], in_=rowmaj0(obs_d))
            m_t = stat.tile([P, TF // P], F32, tag="mf", name="m_t")
            nc.sync.dma_start(out=m_t[:], in_=rowmaj0(maskf_d))
            nc.gpsimd.collective_compute(
                "AllReduce", ALU.add,
                ins=[ws_loc_d.ap()], outs=[ws_all_d.ap()], replica_groups=rg,
            )
            w_t = stat.tile([P, TF // P], F32, tag="wfin", name="w_t")
            nc.sync.dma_start(out=w_t[:], in_=rowmaj0(ws_all_d))
            zg = smal.tile([1, 1], F32, tag="zg", name="zg")
            nc.sync.dma_start(
                out=zg[:],
                in_=ws_all_d.ap()[TF:TF + 1].rearrange("(a n) -> a n", a=1),
            )
            rzg = smal.tile([1, 1], F32, tag="rzg", name="rzg")
            nc.vector.reciprocal(rzg[:], zg[:])
            qfin = smal.tile([1, 1], F32, tag="qfin", name="qfin")
            nc.vector.tensor_scalar_mul(qfin[:], rzg[:], float(c1 * sigma_i))
            qps = psum.tile([P, 1], F32, tag="qps", bufs=1, name="qps")
            nc.tensor.matmul(qps[:], lhsT=onesr[:], rhs=qfin[:], start=True,
                             stop=True)
            qb = smal.tile([P, 1], F32, tag="qb", name="qb")
            nc.vector.tensor_copy(qb[:], qps[:])
            t1 = stat.tile([P, TF // P], F32, tag="t1", name="t1")
            nc.vector.tensor_single_scalar(
                out=t1[:], in_=w_t[:], scalar=qb[:], op=ALU.mult
            )
            t2 = stat.tile([P, TF // P], F32, tag="t2", name="t2")
            nc.vector.tensor_tensor(t2[:], obs_t[:], t1[:], ALU.subtract)
            t3 = stat.tile([P, TF // P], F32, tag="t3", name="t3")
            nc.vector.tensor_tensor(t3[:], t2[:], m_t[:], ALU.mult)
            o_t = stat.tile([P, TF // P], F32, tag="ot", name="o_t")
            nc.vector.tensor_tensor(o_t[:], t1[:], t3[:], ALU.add)
            nc.sync.dma_start(out=rowmaj0(out_d), in_=o_t[:])

    nc.compile()
    return nc


_CACHE: dict = {}
TRACE = False
STAGE = 4
LAST_RESULTS = None


def kernel(Xbar_i, observed_data, time_points, mask, eps, deg_a, deg_b, i):
    global LAST_RESULTS
    i = int(i)
    sigma_i, c1 = _schedule_scalars(i)
    key = ("v2", i, STAGE)
    if key not in _CACHE:
        _CACHE[key] = _build(float(sigma_i), float(c1), stage=STAGE)
    nc = _CACHE[key]

    inv_sig = np.float32(1.0) / sigma_i
    Xb = np.asarray(Xbar_i, np.float32)
    obs = np.asarray(observed_data, np.float32)
    msk = np.asarray(mask, bool)
    tp = np.asarray(time_points, np.float32)
    da = np.asarray(deg_a, np.float32)
    db = np.asarray(deg_b, np.float32)
    epsf = np.asarray(eps, np.float32)

    pred = da[None, :] + db[None, :] * tp[:, None]
    c0 = (Xb * inv_sig).astype(np.float32)
    c0 = np.where(msk, np.float32(1e6), c0).reshape(-1)
    qp = (pred * inv_sig).astype(np.float32)
    qp = np.where(msk, inv_sig, qp).reshape(-1).astype(np.float16)
    obsf = obs.reshape(-1)
    maskf = msk.astype(np.float32).reshape(-1)

    in_maps = []
    for c in range(N_CORES):
        shard = np.ascontiguousarray(
            epsf[c * NLOC:(c + 1) * NLOC].reshape(NLOC, TF)
        )
        in_maps.append(
            {"eps": shard, "c0": c0, "qp": qp, "obs": obsf, "maskf": maskf,
             "ones": np.ones(128, np.float32)}
        )
    kr = run_bass_kernel_spmd(nc, in_maps, list(range(N_CORES)), trace=TRACE)
    LAST_RESULTS = kr
    return kr.results[0]["out"].reshape(T, F).astype(np.float32)


# revision 31
# speedup vs baseline: 1.0108x; 1.0108x over previous
"""MBD degradation-imputation sampling step on 8 Trainium2 NeuronCores.

Strategy (data-parallel over the N=2048 candidate samples, 256/core):
  pass A : per-sample consistency scores, one HBM pass over eps.
           Observed positions are made sample-independent by a
           host-prepared additive tensor c0 that saturates the clip
           (softmax is shift-invariant so the constant contribution
           cancels):
               u  = eps + c0            (DVE tensor_tensor, f32)
               vh = clip(u, +-1/sigma)  (DVE tensor_scalar -> fp16 CACHE)
               d  = vh - q'             (Pool tensor_tensor, fp16)
               S += sum(d^2)            (ACT Square + accum_out)
           The fp16 clipped values stay resident in SBUF (128 KiB/pn)
           so pass B never re-reads eps.
  AllGather the 2048 scores (8 KB), softmax stats on-device, each core
  weights its local samples.
  pass B : weighted partition-reduction straight out of the fp16 SBUF
           cache on the TensorEngine (fp16 matmuls, M=1, PSUM-
           accumulated), AllReduce the (T,F) partials (128 KB), final
           mask-select against observed_data.

`stage` truncates the program for hardware bisection:
  1 = pass A only, 2 = +AllGather/softmax, 3 = +pass B (no AllReduce),
  4 = full kernel.
"""

from contextlib import ExitStack

import numpy as np

import concourse.bass as bass
import concourse.tile as tile
from concourse import bacc, mybir
from concourse.bass_utils import run_bass_kernel_spmd

N_CORES = 8
N, T, F = 2048, 512, 64
P = 128
TF = T * F                      # 32768
NLOC = N // N_CORES             # 256
NBLK = NLOC // P                # 2
CHUNK = 1024
NCHUNK = TF // CHUNK            # 32
SUB = 512                       # matmul N (one PSUM bank)
TEMP = 0.1
T_STEPS = 1000

F32 = mybir.dt.float32
F16 = mybir.dt.float16
AX = mybir.AxisListType
ALU = mybir.AluOpType
ACTF = mybir.ActivationFunctionType


def _schedule_scalars(i: int):
    s = 0.008
    x = np.linspace(0, T_STEPS, T_STEPS + 1, dtype=np.float64)
    ac = np.cos((x / T_STEPS + s) / (1 + s) * np.pi * 0.5) ** 2
    ac = ac / ac[0]
    betas = np.clip(1.0 - ac[1:] / ac[:-1], 0.0, 0.999)
    alphas = 1.0 - betas
    acp = np.cumprod(alphas)
    abar_i = np.float32(acp[i])
    sigma_i = np.float32(np.sqrt(1.0 - acp[i]))
    alpha_i = np.float32(alphas[i])
    abar_im1 = np.float32(acp[i - 1])
    sa = np.float32(np.sqrt(abar_i))
    # the reference's Yi terms cancel exactly; out_missing = c1 * weighted
    c1 = np.float32(sa / np.float32(np.sqrt(alpha_i)) / np.float32(np.sqrt(abar_im1)))
    return sigma_i, c1


def _build(sigma_i: float, c1: float, stage: int = 4):
    inv_sig = float(np.float32(1.0 / np.float32(sigma_i)))
    sigma_i = float(np.float32(sigma_i))
    c1 = float(np.float32(c1))
    # scores = cA * sum((v - q')^2)  (+ sample-independent shift vs ref)
    cA = float(np.float32(-(np.float32(sigma_i) ** 2) / np.float32(TF)))

    nc = bacc.Bacc(
        "TRN2", target_bir_lowering=False, debug=False, num_devices=N_CORES
    )
    eps_d = nc.dram_tensor("eps", [NLOC, TF], F32, kind="ExternalInput")
    c0_d = nc.dram_tensor("c0", [TF], F32, kind="ExternalInput")
    qp_d = nc.dram_tensor("qp", [TF], F16, kind="ExternalInput")
    obs_d = nc.dram_tensor("obs", [TF], F32, kind="ExternalInput")
    maskf_d = nc.dram_tensor("maskf", [TF], F32, kind="ExternalInput")
    out_d = nc.dram_tensor("out", [TF], F32, kind="ExternalOutput")

    ones_d = nc.dram_tensor("ones", [P], F32, kind="ExternalInput")
    sc_loc_d = nc.dram_tensor("sc_loc", [NLOC], F32)
    sc_all_d = nc.dram_tensor("sc_all", [N], F32, addr_space="Shared")
    # ws carries the TF weighted partials plus the local softmax
    # normalizer Z in slot TF — one AllReduce delivers both.
    ws_loc_d = nc.dram_tensor("ws_loc", [TF + 4], F32)
    ws_all_d = nc.dram_tensor("ws_all", [TF + 4], F32, addr_space="Shared")

    rg = [list(range(N_CORES))]

    with tile.TileContext(nc) as tc, ExitStack() as ctx:
        eps_ap = eps_d.ap()

        rowsq = ctx.enter_context(tc.tile_pool(name="rowsq", bufs=4))
        work = ctx.enter_context(tc.tile_pool(name="work", bufs=4))
        workh = ctx.enter_context(tc.tile_pool(name="workh", bufs=2))
        cache = ctx.enter_context(tc.tile_pool(name="cache", bufs=1))
        stat = ctx.enter_context(tc.tile_pool(name="stat", bufs=1))
        smal = ctx.enter_context(tc.tile_pool(name="smal", bufs=1))
        psum = ctx.enter_context(tc.tile_pool(name="psum", bufs=1, space="PSUM"))

        # fp16 clipped-values cache: 64 tiles of [128, 1024] packed into
        # one persistent tile (128 KiB per partition)
        vcache = cache.tile([P, NBLK * NCHUNK * CHUNK], F16, tag="vc",
                            name="vcache")

        # ---------------- pass A: local scores ----------------
        # block-major so block 0's scores can AllGather while block 1
        # computes.  u = c0 + eps is built entirely by DMA engines:
        # ACT-queue broadcast fill of c0, then gpsimd DMA-accumulate eps.
        sa_cols = [
            stat.tile([P, NCHUNK], F32, tag=f"sa{b}", name=f"sa_cols{b}")
            for b in range(NBLK)
        ]
        sb_cols = [
            stat.tile([P, NCHUNK], F32, tag=f"sb{b}", name=f"sb_cols{b}")
            for b in range(NBLK)
        ]
        s_loc = stat.tile([P, NBLK], F32, tag="sloc", name="s_loc")
        for b in range(NBLK):
            for k in range(NCHUNK):
                sl = slice(k * CHUNK, (k + 1) * CHUNK)
                q_t = rowsq.tile([P, CHUNK], F16, tag="q", name="q_t")
                nc.gpsimd.dma_start(
                    out=q_t[:], in_=qp_d.ap()[sl].partition_broadcast(P)
                )
                c0_t = rowsq.tile([P, CHUNK], F32, tag="c0", name="c0_t")
                c0eng = nc.sync if k % 2 == 0 else nc.gpsimd
                c0eng.dma_start(
                    out=c0_t[:], in_=c0_d.ap()[sl].partition_broadcast(P)
                )
                u_t = work.tile([P, CHUNK], F32, tag="u", name="u_t")
                nc.sync.dma_start(out=u_t[:], in_=eps_ap[b * P:(b + 1) * P, sl])
                nc.vector.scalar_tensor_tensor(
                    out=u_t[:], in0=u_t[:], scalar=0.0, in1=c0_t[:],
                    op0=ALU.add, op1=ALU.add,
                )
                off = (k * NBLK + b) * CHUNK
                vsl = vcache[:, off:off + CHUNK]
                nc.vector.tensor_scalar(
                    out=vsl, in0=u_t[:], scalar1=inv_sig, scalar2=-inv_sig,
                    op0=ALU.min, op1=ALU.max,
                )
                d_t = workh.tile([P, CHUNK], F16, tag="d", name="d_t")
                nc.vector.scalar_tensor_tensor(
                    out=d_t[:], in0=vsl, scalar=1.0, in1=q_t[:],
                    op0=ALU.mult, op1=ALU.mult,
                    accum_out=sb_cols[b][:, k:k + 1],
                )
                d2 = workh.tile([P, CHUNK], F16, tag="d", name="d2")
                nc.scalar.activation(
                    out=d2[:], in_=vsl, func=ACTF.Square,
                    accum_out=sa_cols[b][:, k:k + 1],
                )
            # block-b scores -> DRAM -> AllGather (overlaps next block)
            sa_tot = smal.tile([P, 1], F32, tag="sat", name="sa_tot")
            nc.vector.tensor_reduce(sa_tot[:], sa_cols[b][:], axis=AX.X, op=ALU.add)
            sb_tot = smal.tile([P, 1], F32, tag="sbt", name="sb_tot")
            nc.vector.tensor_reduce(sb_tot[:], sb_cols[b][:], axis=AX.X, op=ALU.add)
            dtot = smal.tile([P, 1], F32, tag="dtot", name="dtot")
            nc.vector.scalar_tensor_tensor(
                out=dtot[:], in0=sb_tot[:], scalar=-2.0, in1=sa_tot[:],
                op0=ALU.mult, op1=ALU.add,
            )
            nc.vector.tensor_scalar_mul(s_loc[:, b:b + 1], dtot[:], cA)
            nc.sync.dma_start(
                out=sc_loc_d.ap()[b * P:(b + 1) * P]
                .rearrange("(a p) -> p a", a=1),
                in_=s_loc[:, b:b + 1],
            )
            if stage >= 2:
                nc.gpsimd.collective_compute(
                    "AllGather", ALU.bypass,
                    ins=[sc_loc_d.ap()[b * P:(b + 1) * P]],
                    outs=[sc_all_d.ap()[b * P * N_CORES:(b + 1) * P * N_CORES]],
                    replica_groups=rg,
                )
        if stage <= 1:
            nc.sync.dma_start(
                out=out_d.ap()[0:NLOC].rearrange("(b p) -> p b", p=P),
                in_=s_loc[:],
            )

        # ---------------- softmax stats ----------------
        # weights are UN-normalized exp() here; the global Z rides the
        # AllReduce (slot TF of ws) and division happens post-reduce.
        wt16 = None
        if stage >= 2:
            onesr = smal.tile([1, P], F32, tag="onesr", name="onesr")
            nc.sync.dma_start(
                out=onesr[:], in_=ones_d.ap().rearrange("(a n) -> a n", a=1)
            )
            onec = smal.tile([P, 1], F32, tag="onec", name="onec")
            nc.sync.dma_start(
                out=onec[:], in_=ones_d.ap().rearrange("(p a) -> p a", a=1)
            )
            s_all = smal.tile([1, N], F32, tag="sall", name="s_all")
            nc.sync.dma_start(
                out=s_all[:], in_=sc_all_d.ap().rearrange("(a n) -> a n", a=1)
            )
            pack = smal.tile([1, 2], F32, tag="pack", name="pack")
            negmean = smal.tile([1, 1], F32, tag="negmean", name="negmean")
            nc.vector.tensor_reduce(negmean[:], s_all[:], axis=AX.X, op=ALU.add)
            nc.vector.tensor_scalar_mul(negmean[:], negmean[:], -1.0 / N)
            js = smal.tile([1, N], F16, tag="js", name="js")
            ssq = smal.tile([1, 1], F32, tag="ssq", name="ssq")
            nc.scalar.activation(
                out=js[:], in_=s_all[:], func=ACTF.Square, bias=negmean[:],
                accum_out=ssq[:],
            )
            # std = max(sqrt(ssq/(N-1)), 1e-4); pack0 = 1/(std*TEMP)
            std = smal.tile([1, 1], F32, tag="std", name="std")
            nc.scalar.activation(
                out=std[:], in_=ssq[:], func=ACTF.Sqrt, scale=1.0 / (N - 1)
            )
            stdT = smal.tile([1, 1], F32, tag="stdT", name="stdT")
            nc.vector.tensor_scalar(
                out=stdT[:], in0=std[:], scalar1=1e-4, scalar2=TEMP,
                op0=ALU.max, op1=ALU.mult,
            )
            nc.vector.reciprocal(pack[:, 0:1], stdT[:])
            mx = smal.tile([1, 1], F32, tag="mx", name="mx")
            nc.vector.tensor_reduce(mx[:], s_all[:], axis=AX.X, op=ALU.max)
            # shifted logit: (s - mx)*inv10 (mean cancels in the shift, and
            # the un-normalized exp is safe: max exponent is exactly 0)
            nmx = smal.tile([1, 1], F32, tag="nmx", name="nmx")
            nc.vector.tensor_scalar_mul(nmx[:], mx[:], -1.0)
            nc.vector.tensor_tensor(pack[:, 1:2], nmx[:], pack[:, 0:1], ALU.mult)
            # PE-broadcast (inv10, bg) to all 128 partitions
            bps = psum.tile([P, 2], F32, tag="bps", bufs=1, name="bps")
            nc.tensor.matmul(bps[:], lhsT=onesr[:], rhs=pack[:], start=True,
                             stop=True)
            scal = smal.tile([P, 2], F32, tag="scal", name="scal")
            nc.vector.tensor_copy(scal[:], bps[:])

            e_loc = smal.tile([P, NBLK], F32, tag="eloc", name="e_loc")
            nc.scalar.activation(
                out=e_loc[:], in_=s_loc[:], func=ACTF.Exp,
                scale=scal[:, 0:1], bias=scal[:, 1:2],
            )
            wt16 = stat.tile([P, NBLK], F16, tag="wt16", name="wt16")
            zloc = smal.tile([P, 1], F32, tag="zloc", name="zloc")
            nc.scalar.activation(
                out=wt16[:], in_=e_loc[:], func=ACTF.Copy, accum_out=zloc[:]
            )
            # local Z -> ws_loc[TF] so the AllReduce sums it globally
            zpt = psum.tile([P, 1], F32, tag="qps", bufs=1, name="zpt")
            zps = zpt[0:1, 0:1]
            nc.tensor.matmul(zps, lhsT=zloc[:], rhs=onec[:], start=True,
                             stop=True)
            ztot = smal.tile([1, 1], F32, tag="ztot", name="ztot")
            nc.vector.tensor_copy(ztot[:], zps[:])
            nc.sync.dma_start(
                out=ws_loc_d.ap()[TF:TF + 1].rearrange("(a n) -> a n", a=1),
                in_=ztot[:],
            )
            if stage <= 2:
                nc.sync.dma_start(
                    out=out_d.ap()[0:NLOC].rearrange("(b p) -> p b", p=P),
                    in_=e_loc[:],
                )

        # ---------------- pass B: weighted sum on PE from SBUF cache ----
        if stage >= 3:
            # warm the PE p-state during the stats window: dummy matmuls
            # gated on a post-gather tile so they run right before pass B
            jl = smal.tile([P, 1], F16, tag="jl", name="jl")
            nc.scalar.copy(jl[:], s_loc[:, 0:1])
            for w in range(10):
                wps = psum.tile([1, SUB], F32, tag="wrow", bufs=6, name="wps")
                nc.tensor.matmul(
                    wps[:], lhsT=jl[:], rhs=vcache[:, w * SUB:(w + 1) * SUB],
                    start=True, stop=True,
                )
            GRP = 2
            for s in range(TF // SUB):
                k, half = s // 2, s % 2
                wrow = psum.tile([1, SUB], F32, tag="wrow", bufs=6, name="wrow")
                for b in range(NBLK):
                    off = (k * NBLK + b) * CHUNK + half * SUB
                    nc.tensor.matmul(
                        wrow[:], lhsT=wt16[:, b:b + 1],
                        rhs=vcache[:, off:off + SUB],
                        start=(b == 0), stop=(b == NBLK - 1),
                    )
                wsb = work.tile([1, SUB], F32, tag="wsb", bufs=6, name="wsb")
                if s % 2 == 0:
                    nc.vector.tensor_copy(wsb[:], wrow[:])
                else:
                    nc.scalar.copy(wsb[:], wrow[:])
                nc.sync.dma_start(
                    out=ws_loc_d.ap()[s * SUB:(s + 1) * SUB]
                    .rearrange("(a n) -> a n", a=1),
                    in_=wsb[:],
                )
            if stage <= 3:
                o3 = stat.tile([P, TF // P], F32, tag="o3", name="o3")
                nc.sync.dma_start(
                    out=o3[:],
                    in_=ws_loc_d.ap()[0:TF].rearrange("(p c) -> p c", p=P),
                )
                nc.sync.dma_start(
                    out=out_d.ap().rearrange("(p c) -> p c", p=P), in_=o3[:]
                )

        # ---------------- AllReduce + final combine ----------------
        if stage >= 4:
            # obs/mask preloads don't depend on anything — issue early is
            # handled by the scheduler; they're plain loads.
            rowmaj0 = lambda d: d.ap()[0:TF].rearrange("(p c) -> p c", p=P)
            obs_t = stat.tile([P, TF // P], F32, tag="obsf", name="obs_t")
            nc.sync.dma_start(out=obs_t[:], in_=rowmaj0(obs_d))
            m_t = stat.tile([P, TF // P], F32, tag="mf", name="m_t")
            nc.sync.dma_start(out=m_t[:], in_=rowmaj0(maskf_d))
            nc.gpsimd.collective_compute(
                "AllReduce", ALU.add,
                ins=[ws_loc_d.ap()], outs=[ws_all_d.ap()], replica_groups=rg,
            )
            w_t = stat.tile([P, TF // P], F32, tag="wfin", name="w_t")
            nc.sync.dma_start(out=w_t[:], in_=rowmaj0(ws_all_d))
            zg = smal.tile([1, 1], F32, tag="zg", name="zg")
            nc.sync.dma_start(
                out=zg[:],
                in_=ws_all_d.ap()[TF:TF + 1].rearrange("(a n) -> a n", a=1),
            )
            rzg = smal.tile([1, 1], F32, tag="rzg", name="rzg")
            nc.vector.reciprocal(rzg[:], zg[:])
            qfin = smal.tile([1, 1], F32, tag="qfin", name="qfin")
            nc.vector.tensor_scalar_mul(qfin[:], rzg[:], float(c1 * sigma_i))
            qps = psum.tile([P, 1], F32, tag="qps", bufs=1, name="qps")
            nc.tensor.matmul(qps[:], lhsT=onesr[:], rhs=qfin[:], start=True,
                             stop=True)
            qb = smal.tile([P, 1], F32, tag="qb", name="qb")
            nc.vector.tensor_copy(qb[:], qps[:])
            t1 = stat.tile([P, TF // P], F32, tag="t1", name="t1")
            nc.vector.tensor_single_scalar(
                out=t1[:], in_=w_t[:], scalar=qb[:], op=ALU.mult
            )
            t2 = stat.tile([P, TF // P], F32, tag="t2", name="t2")
            nc.vector.tensor_tensor(t2[:], obs_t[:], t1[:], ALU.subtract)
            t3 = stat.tile([P, TF // P], F32, tag="t3", name="t3")
            nc.vector.tensor_tensor(t3[:], t2[:], m_t[:], ALU.mult)
            o_t = stat.tile([P, TF // P], F32, tag="ot", name="o_t")
            nc.vector.tensor_tensor(o_t[:], t1[:], t3[:], ALU.add)
            nc.sync.dma_start(out=rowmaj0(out_d), in_=o_t[:])

    nc.compile()
    return nc


_CACHE: dict = {}
TRACE = False
STAGE = 4
LAST_RESULTS = None


def kernel(Xbar_i, observed_data, time_points, mask, eps, deg_a, deg_b, i):
    global LAST_RESULTS
    i = int(i)
    sigma_i, c1 = _schedule_scalars(i)
    key = ("v2", i, STAGE)
    if key not in _CACHE:
        _CACHE[key] = _build(float(sigma_i), float(c1), stage=STAGE)
    nc = _CACHE[key]

    inv_sig = np.float32(1.0) / sigma_i
    Xb = np.asarray(Xbar_i, np.float32)
    obs = np.asarray(observed_data, np.float32)
    msk = np.asarray(mask, bool)
    tp = np.asarray(time_points, np.float32)
    da = np.asarray(deg_a, np.float32)
    db = np.asarray(deg_b, np.float32)
    epsf = np.asarray(eps, np.float32)

    pred = da[None, :] + db[None, :] * tp[:, None]
    c0 = (Xb * inv_sig).astype(np.float32)
    c0 = np.where(msk, np.float32(1e6), c0).reshape(-1)
    qp = (pred * inv_sig).astype(np.float32)
    qp = np.where(msk, inv_sig, qp).reshape(-1).astype(np.float16)
    obsf = obs.reshape(-1)
    maskf = msk.astype(np.float32).reshape(-1)

    in_maps = []
    for c in range(N_CORES):
        shard = np.ascontiguousarray(
            epsf[c * NLOC:(c + 1) * NLOC].reshape(NLOC, TF)
        )
        in_maps.append(
            {"eps": shard, "c0": c0, "qp": qp, "obs": obsf, "maskf": maskf,
             "ones": np.ones(128, np.float32)}
        )
    kr = run_bass_kernel_spmd(nc, in_maps, list(range(N_CORES)), trace=TRACE)
    LAST_RESULTS = kr
    return kr.results[0]["out"].reshape(T, F).astype(np.float32)


# revision 33
# speedup vs baseline: 1.1313x; 1.1193x over previous
"""MBD degradation-imputation sampling step on 8 Trainium2 NeuronCores.

Strategy (data-parallel over the N=2048 candidate samples, 256/core):
  pass A : per-sample consistency scores, one HBM pass over eps.
           Observed positions are made sample-independent by a
           host-prepared additive tensor c0 that saturates the clip
           (softmax is shift-invariant so the constant contribution
           cancels):
               u  = eps + c0            (DVE tensor_tensor, f32)
               vh = clip(u, +-1/sigma)  (DVE tensor_scalar -> fp16 CACHE)
               d  = vh - q'             (Pool tensor_tensor, fp16)
               S += sum(d^2)            (ACT Square + accum_out)
           The fp16 clipped values stay resident in SBUF (128 KiB/pn)
           so pass B never re-reads eps.
  AllGather the 2048 scores (8 KB), softmax stats on-device, each core
  weights its local samples.
  pass B : weighted partition-reduction straight out of the fp16 SBUF
           cache on the TensorEngine (fp16 matmuls, M=1, PSUM-
           accumulated), AllReduce the (T,F) partials (128 KB), final
           mask-select against observed_data.

`stage` truncates the program for hardware bisection:
  1 = pass A only, 2 = +AllGather/softmax, 3 = +pass B (no AllReduce),
  4 = full kernel.
"""

from contextlib import ExitStack

import numpy as np

import concourse.bass as bass
import concourse.tile as tile
from concourse import bacc, mybir
from concourse.bass_utils import run_bass_kernel_spmd

N_CORES = 8
N, T, F = 2048, 512, 64
P = 128
TF = T * F                      # 32768
NLOC = N // N_CORES             # 256
NBLK = NLOC // P                # 2
CHUNK = 1024
NCHUNK = TF // CHUNK            # 32
SUB = 512                       # matmul N (one PSUM bank)
TEMP = 0.1
T_STEPS = 1000

F32 = mybir.dt.float32
F16 = mybir.dt.float16
AX = mybir.AxisListType
ALU = mybir.AluOpType
ACTF = mybir.ActivationFunctionType


def _schedule_scalars(i: int):
    s = 0.008
    x = np.linspace(0, T_STEPS, T_STEPS + 1, dtype=np.float64)
    ac = np.cos((x / T_STEPS + s) / (1 + s) * np.pi * 0.5) ** 2
    ac = ac / ac[0]
    betas = np.clip(1.0 - ac[1:] / ac[:-1], 0.0, 0.999)
    alphas = 1.0 - betas
    acp = np.cumprod(alphas)
    abar_i = np.float32(acp[i])
    sigma_i = np.float32(np.sqrt(1.0 - acp[i]))
    alpha_i = np.float32(alphas[i])
    abar_im1 = np.float32(acp[i - 1])
    sa = np.float32(np.sqrt(abar_i))
    # the reference's Yi terms cancel exactly; out_missing = c1 * weighted
    c1 = np.float32(sa / np.float32(np.sqrt(alpha_i)) / np.float32(np.sqrt(abar_im1)))
    return sigma_i, c1


def _build(sigma_i: float, c1: float, stage: int = 4):
    inv_sig = float(np.float32(1.0 / np.float32(sigma_i)))
    sigma_i = float(np.float32(sigma_i))
    c1 = float(np.float32(c1))
    # scores = cA * sum((v - q')^2)  (+ sample-independent shift vs ref)
    cA = float(np.float32(-(np.float32(sigma_i) ** 2) / np.float32(TF)))

    nc = bacc.Bacc(
        "TRN2", target_bir_lowering=False, debug=False, num_devices=N_CORES
    )
    eps_d = nc.dram_tensor("eps", [NLOC, TF], F32, kind="ExternalInput")
    c0_d = nc.dram_tensor("c0", [TF], F32, kind="ExternalInput")
    qp_d = nc.dram_tensor("qp", [TF], F16, kind="ExternalInput")
    obs_d = nc.dram_tensor("obs", [TF], F32, kind="ExternalInput")
    maskf_d = nc.dram_tensor("maskf", [TF], F32, kind="ExternalInput")
    out_d = nc.dram_tensor("out", [TF], F32, kind="ExternalOutput")

    ones_d = nc.dram_tensor("ones", [P], F32, kind="ExternalInput")
    sc_loc_d = nc.dram_tensor("sc_loc", [NLOC], F32)
    sc_all_d = nc.dram_tensor("sc_all", [N], F32, addr_space="Shared")
    # ws carries the TF weighted partials plus the local softmax
    # normalizer Z in slot TF — one AllReduce delivers both.
    ws_loc_d = nc.dram_tensor("ws_loc", [TF + 4], F32)
    ws_all_d = nc.dram_tensor("ws_all", [TF + 4], F32, addr_space="Shared")

    rg = [list(range(N_CORES))]

    with tile.TileContext(nc) as tc, ExitStack() as ctx:
        eps_ap = eps_d.ap()

        rowsq = ctx.enter_context(tc.tile_pool(name="rowsq", bufs=4))
        work = ctx.enter_context(tc.tile_pool(name="work", bufs=4))
        workh = ctx.enter_context(tc.tile_pool(name="workh", bufs=2))
        cache = ctx.enter_context(tc.tile_pool(name="cache", bufs=1))
        stat = ctx.enter_context(tc.tile_pool(name="stat", bufs=1))
        smal = ctx.enter_context(tc.tile_pool(name="smal", bufs=1))
        psum = ctx.enter_context(tc.tile_pool(name="psum", bufs=1, space="PSUM"))

        # fp16 clipped-values cache: 64 tiles of [128, 1024] packed into
        # one persistent tile (128 KiB per partition)
        vcache = cache.tile([P, NBLK * NCHUNK * CHUNK], F16, tag="vc",
                            name="vcache")

        # ---------------- pass A: local scores ----------------
        # block-major so block 0's scores can AllGather while block 1
        # computes.  u = c0 + eps is built entirely by DMA engines:
        # ACT-queue broadcast fill of c0, then gpsimd DMA-accumulate eps.
        sa_cols = [
            stat.tile([P, NCHUNK], F32, tag=f"sa{b}", name=f"sa_cols{b}")
            for b in range(NBLK)
        ]
        sb_cols = [
            stat.tile([P, NCHUNK], F32, tag=f"sb{b}", name=f"sb_cols{b}")
            for b in range(NBLK)
        ]
        s_loc = stat.tile([P, NBLK], F32, tag="sloc", name="s_loc")
        for b in range(NBLK):
            for k in range(NCHUNK):
                sl = slice(k * CHUNK, (k + 1) * CHUNK)
                q_t = rowsq.tile([P, CHUNK], F16, tag="q", name="q_t")
                nc.gpsimd.dma_start(
                    out=q_t[:], in_=qp_d.ap()[sl].partition_broadcast(P)
                )
                c0_t = rowsq.tile([P, CHUNK], F32, tag="c0", name="c0_t")
                nc.scalar.dma_start(
                    out=c0_t[:], in_=c0_d.ap()[sl].partition_broadcast(P)
                )
                u_t = work.tile([P, CHUNK], F32, tag="u", name="u_t")
                nc.sync.dma_start(out=u_t[:], in_=eps_ap[b * P:(b + 1) * P, sl])
                nc.vector.scalar_tensor_tensor(
                    out=u_t[:], in0=u_t[:], scalar=0.0, in1=c0_t[:],
                    op0=ALU.add, op1=ALU.add,
                )
                off = (k * NBLK + b) * CHUNK
                vsl = vcache[:, off:off + CHUNK]
                nc.vector.tensor_scalar(
                    out=vsl, in0=u_t[:], scalar1=inv_sig, scalar2=-inv_sig,
                    op0=ALU.min, op1=ALU.max,
                )
                d_t = workh.tile([P, CHUNK], F16, tag="d", name="d_t")
                nc.vector.scalar_tensor_tensor(
                    out=d_t[:], in0=vsl, scalar=1.0, in1=q_t[:],
                    op0=ALU.mult, op1=ALU.mult,
                    accum_out=sb_cols[b][:, k:k + 1],
                )
                d2 = workh.tile([P, CHUNK], F16, tag="d", name="d2")
                nc.scalar.activation(
                    out=d2[:], in_=vsl, func=ACTF.Square,
                    accum_out=sa_cols[b][:, k:k + 1],
                )
            # block-b scores -> DRAM -> AllGather (overlaps next block)
            sa_tot = smal.tile([P, 1], F32, tag="sat", name="sa_tot")
            nc.vector.tensor_reduce(sa_tot[:], sa_cols[b][:], axis=AX.X, op=ALU.add)
            sb_tot = smal.tile([P, 1], F32, tag="sbt", name="sb_tot")
            nc.vector.tensor_reduce(sb_tot[:], sb_cols[b][:], axis=AX.X, op=ALU.add)
            dtot = smal.tile([P, 1], F32, tag="dtot", name="dtot")
            nc.vector.scalar_tensor_tensor(
                out=dtot[:], in0=sb_tot[:], scalar=-2.0, in1=sa_tot[:],
                op0=ALU.mult, op1=ALU.add,
            )
            nc.vector.tensor_scalar_mul(s_loc[:, b:b + 1], dtot[:], cA)
            nc.sync.dma_start(
                out=sc_loc_d.ap()[b * P:(b + 1) * P]
                .rearrange("(a p) -> p a", a=1),
                in_=s_loc[:, b:b + 1],
            )
            if stage >= 2:
                nc.gpsimd.collective_compute(
                    "AllGather", ALU.bypass,
                    ins=[sc_loc_d.ap()[b * P:(b + 1) * P]],
                    outs=[sc_all_d.ap()[b * P * N_CORES:(b + 1) * P * N_CORES]],
                    replica_groups=rg,
                )
        if stage <= 1:
            nc.sync.dma_start(
                out=out_d.ap()[0:NLOC].rearrange("(b p) -> p b", p=P),
                in_=s_loc[:],
            )

        # ---------------- softmax stats ----------------
        # weights are UN-normalized exp() here; the global Z rides the
        # AllReduce (slot TF of ws) and division happens post-reduce.
        wt16 = None
        if stage >= 2:
            onesr = smal.tile([1, P], F32, tag="onesr", name="onesr")
            nc.sync.dma_start(
                out=onesr[:], in_=ones_d.ap().rearrange("(a n) -> a n", a=1)
            )
            onec = smal.tile([P, 1], F32, tag="onec", name="onec")
            nc.sync.dma_start(
                out=onec[:], in_=ones_d.ap().rearrange("(p a) -> p a", a=1)
            )
            s_all = smal.tile([1, N], F32, tag="sall", name="s_all")
            nc.sync.dma_start(
                out=s_all[:], in_=sc_all_d.ap().rearrange("(a n) -> a n", a=1)
            )
            pack = smal.tile([1, 2], F32, tag="pack", name="pack")
            negmean = smal.tile([1, 1], F32, tag="negmean", name="negmean")
            nc.vector.tensor_reduce(negmean[:], s_all[:], axis=AX.X, op=ALU.add)
            nc.vector.tensor_scalar_mul(negmean[:], negmean[:], -1.0 / N)
            js = smal.tile([1, N], F16, tag="js", name="js")
            ssq = smal.tile([1, 1], F32, tag="ssq", name="ssq")
            nc.scalar.activation(
                out=js[:], in_=s_all[:], func=ACTF.Square, bias=negmean[:],
                accum_out=ssq[:],
            )
            # std = max(sqrt(ssq/(N-1)), 1e-4); pack0 = 1/(std*TEMP)
            std = smal.tile([1, 1], F32, tag="std", name="std")
            nc.scalar.activation(
                out=std[:], in_=ssq[:], func=ACTF.Sqrt, scale=1.0 / (N - 1)
            )
            stdT = smal.tile([1, 1], F32, tag="stdT", name="stdT")
            nc.vector.tensor_scalar(
                out=stdT[:], in0=std[:], scalar1=1e-4, scalar2=TEMP,
                op0=ALU.max, op1=ALU.mult,
            )
            nc.vector.reciprocal(pack[:, 0:1], stdT[:])
            mx = smal.tile([1, 1], F32, tag="mx", name="mx")
            nc.vector.tensor_reduce(mx[:], s_all[:], axis=AX.X, op=ALU.max)
            # shifted logit: (s - mx)*inv10 (mean cancels in the shift, and
            # the un-normalized exp is safe: max exponent is exactly 0)
            nmx = smal.tile([1, 1], F32, tag="nmx", name="nmx")
            nc.vector.tensor_scalar_mul(nmx[:], mx[:], -1.0)
            nc.vector.tensor_tensor(pack[:, 1:2], nmx[:], pack[:, 0:1], ALU.mult)
            # PE-broadcast (inv10, bg) to all 128 partitions
            bps = psum.tile([P, 2], F32, tag="bps", bufs=1, name="bps")
            nc.tensor.matmul(bps[:], lhsT=onesr[:], rhs=pack[:], start=True,
                             stop=True)
            scal = smal.tile([P, 2], F32, tag="scal", name="scal")
            nc.vector.tensor_copy(scal[:], bps[:])

            e_loc = smal.tile([P, NBLK], F32, tag="eloc", name="e_loc")
            nc.scalar.activation(
                out=e_loc[:], in_=s_loc[:], func=ACTF.Exp,
                scale=scal[:, 0:1], bias=scal[:, 1:2],
            )
            wt16 = stat.tile([P, NBLK], F16, tag="wt16", name="wt16")
            zloc = smal.tile([P, 1], F32, tag="zloc", name="zloc")
            nc.scalar.activation(
                out=wt16[:], in_=e_loc[:], func=ACTF.Copy, accum_out=zloc[:]
            )
            # local Z -> ws_loc[TF] so the AllReduce sums it globally
            zpt = psum.tile([P, 1], F32, tag="qps", bufs=1, name="zpt")
            zps = zpt[0:1, 0:1]
            nc.tensor.matmul(zps, lhsT=zloc[:], rhs=onec[:], start=True,
                             stop=True)
            ztot = smal.tile([1, 1], F32, tag="ztot", name="ztot")
            nc.vector.tensor_copy(ztot[:], zps[:])
            nc.sync.dma_start(
                out=ws_loc_d.ap()[TF:TF + 1].rearrange("(a n) -> a n", a=1),
                in_=ztot[:],
            )
            if stage <= 2:
                nc.sync.dma_start(
                    out=out_d.ap()[0:NLOC].rearrange("(b p) -> p b", p=P),
                    in_=e_loc[:],
                )

        # ---------------- pass B: weighted sum on PE from SBUF cache ----
        if stage >= 3:
            for s in range(TF // SUB):
                k, half = s // 2, s % 2
                wrow = psum.tile([1, SUB], F32, tag="wrow", bufs=6, name="wrow")
                for b in range(NBLK):
                    off = (k * NBLK + b) * CHUNK + half * SUB
                    nc.tensor.matmul(
                        wrow[:], lhsT=wt16[:, b:b + 1],
                        rhs=vcache[:, off:off + SUB],
                        start=(b == 0), stop=(b == NBLK - 1),
                    )
                wsb = work.tile([1, SUB], F32, tag="wsb", bufs=6, name="wsb")
                if s % 2 == 0:
                    nc.vector.tensor_copy(wsb[:], wrow[:])
                else:
                    nc.scalar.copy(wsb[:], wrow[:])
                nc.sync.dma_start(
                    out=ws_loc_d.ap()[s * SUB:(s + 1) * SUB]
                    .rearrange("(a n) -> a n", a=1),
                    in_=wsb[:],
                )
            if stage <= 3:
                o3 = stat.tile([P, TF // P], F32, tag="o3", name="o3")
                nc.sync.dma_start(
                    out=o3[:],
                    in_=ws_loc_d.ap()[0:TF].rearrange("(p c) -> p c", p=P),
                )
                nc.sync.dma_start(
                    out=out_d.ap().rearrange("(p c) -> p c", p=P), in_=o3[:]
                )

        # ---------------- AllReduce + final combine ----------------
        if stage >= 4:
            # obs/mask preloads don't depend on anything — issue early is
            # handled by the scheduler; they're plain loads.
            rowmaj0 = lambda d: d.ap()[0:TF].rearrange("(p c) -> p c", p=P)
            obs_t = stat.tile([P, TF // P], F32, tag="obsf", name="obs_t")
            nc.sync.dma_start(out=obs_t[:], in_=rowmaj0(obs_d))
            m_t = stat.tile([P, TF // P], F32, tag="mf", name="m_t")
            nc.sync.dma_start(out=m_t[:], in_=rowmaj0(maskf_d))
            nc.gpsimd.collective_compute(
                "AllReduce", ALU.add,
                ins=[ws_loc_d.ap()], outs=[ws_all_d.ap()], replica_groups=rg,
            )
            w_t = stat.tile([P, TF // P], F32, tag="wfin", name="w_t")
            nc.sync.dma_start(out=w_t[:], in_=rowmaj0(ws_all_d))
            zg = smal.tile([1, 1], F32, tag="zg", name="zg")
            nc.sync.dma_start(
                out=zg[:],
                in_=ws_all_d.ap()[TF:TF + 1].rearrange("(a n) -> a n", a=1),
            )
            rzg = smal.tile([1, 1], F32, tag="rzg", name="rzg")
            nc.vector.reciprocal(rzg[:], zg[:])
            qfin = smal.tile([1, 1], F32, tag="qfin", name="qfin")
            nc.vector.tensor_scalar_mul(qfin[:], rzg[:], float(c1 * sigma_i))
            qps = psum.tile([P, 1], F32, tag="qps", bufs=1, name="qps")
            nc.tensor.matmul(qps[:], lhsT=onesr[:], rhs=qfin[:], start=True,
                             stop=True)
            qb = smal.tile([P, 1], F32, tag="qb", name="qb")
            nc.vector.tensor_copy(qb[:], qps[:])
            t1 = stat.tile([P, TF // P], F32, tag="t1", name="t1")
            nc.vector.tensor_single_scalar(
                out=t1[:], in_=w_t[:], scalar=qb[:], op=ALU.mult
            )
            t2 = stat.tile([P, TF // P], F32, tag="t2", name="t2")
            nc.vector.tensor_tensor(t2[:], obs_t[:], t1[:], ALU.subtract)
            t3 = stat.tile([P, TF // P], F32, tag="t3", name="t3")
            nc.vector.tensor_tensor(t3[:], t2[:], m_t[:], ALU.mult)
            o_t = stat.tile([P, TF // P], F32, tag="ot", name="o_t")
            nc.vector.tensor_tensor(o_t[:], t1[:], t3[:], ALU.add)
            nc.sync.dma_start(out=rowmaj0(out_d), in_=o_t[:])

    nc.compile()
    return nc


_CACHE: dict = {}
TRACE = False
STAGE = 4
LAST_RESULTS = None


def kernel(Xbar_i, observed_data, time_points, mask, eps, deg_a, deg_b, i):
    global LAST_RESULTS
    i = int(i)
    sigma_i, c1 = _schedule_scalars(i)
    key = ("v2", i, STAGE)
    if key not in _CACHE:
        _CACHE[key] = _build(float(sigma_i), float(c1), stage=STAGE)
    nc = _CACHE[key]

    inv_sig = np.float32(1.0) / sigma_i
    Xb = np.asarray(Xbar_i, np.float32)
    obs = np.asarray(observed_data, np.float32)
    msk = np.asarray(mask, bool)
    tp = np.asarray(time_points, np.float32)
    da = np.asarray(deg_a, np.float32)
    db = np.asarray(deg_b, np.float32)
    epsf = np.asarray(eps, np.float32)

    pred = da[None, :] + db[None, :] * tp[:, None]
    c0 = (Xb * inv_sig).astype(np.float32)
    c0 = np.where(msk, np.float32(1e6), c0).reshape(-1)
    qp = (pred * inv_sig).astype(np.float32)
    qp = np.where(msk, inv_sig, qp).reshape(-1).astype(np.float16)
    obsf = obs.reshape(-1)
    maskf = msk.astype(np.float32).reshape(-1)

    in_maps = []
    for c in range(N_CORES):
        shard = np.ascontiguousarray(
            epsf[c * NLOC:(c + 1) * NLOC].reshape(NLOC, TF)
        )
        in_maps.append(
            {"eps": shard, "c0": c0, "qp": qp, "obs": obsf, "maskf": maskf,
             "ones": np.ones(128, np.float32)}
        )
    kr = run_bass_kernel_spmd(nc, in_maps, list(range(N_CORES)), trace=TRACE)
    LAST_RESULTS = kr
    return kr.results[0]["out"].reshape(T, F).astype(np.float32)
